# revision 22
# baseline (speedup 1.0000x reference)
"""Trainium2 Bass kernel for nn_Message_Passer (gnn_message_passing).

Reference computation:
    A = relu(edge_ij @ W + b)            # [B, E, 1024]
    messages = einsum("beij,bej->bei", A.reshape(B,E,32,32), node_j)

Strategy (8 NeuronCores, data-parallel over the flattened B*E edge dim):
  - Host pre-transposes inputs: edgeT_aug [65, BE] (64 edge features + ones row
    so the bias rides inside the matmul), nodeT [32, BE] (bf16), W_aug [65,1024].
  - matmul1 (PE, float32r single-pass mode): lhsT = W_aug column-block g,
    rhs = edgeT tile -> AT_g [128, ET] in PSUM. Partition p of bank g is
    A-column k = 128g + p, i.e. (i, j) = (k // 32, k % 32).
  - Fused relu+multiply: P = max(AT, 0) * nodeT_rep, where nodeT_rep[p, e] =
    node[e, p % 32] (a 4x-replicated [128, *] bf16 tile serves every bank).
    Done with DVE scalar_tensor_tensor straight out of PSUM; a fraction of
    bank-pairs instead goes ACT relu (PSUM->SBUF bf16) + DVE tensor_tensor at
    2x so the work splits across both engines.
  - j-reduction via PE: constant 0/1 selector matmuls accumulate
    sum_j P[(i,j), e] into PSUM. Selector block (q, d) maps bank g = 2q+d
    partitions p to output row 32c + 8q + 4d + p//32 = 32c + i, where
    c = tile%4 selects the strip via tile_position, so FOUR tiles pack one
    msg PSUM bank [128, ET] fully (row 32c + i <-> tile 4k+c, msg row i).
  - One PSUM->SBUF msg eviction + one DMA per 4 tiles (full 128-row banks,
    4x less evict work and 4x less output DMA than per-tile strips).
  - Host extracts msg[tile 4k+c][e, i] = raw_k[32c + i, e].
"""

import threading

import numpy as np
import ml_dtypes

import concourse.bass as bass
import concourse.mybir as mybir
import concourse.tile as tile
from concourse import bacc
from concourse.bass import ts, ds
from concourse.bass_utils import run_bass_kernel_spmd

N_CORES = 8
B, E_FULL, ND, ED = 16, 4096, 32, 64
EDGES = B * E_FULL            # 65536
E_CORE = EDGES // N_CORES     # 8192
ET = 512                      # edges per on-chip tile
NT = E_CORE // ET             # 16 tiles
GT = 4                        # tiles per input-load group
GRP = GT * ET                 # 2048 edges per load group
KDIM = ED + 1                 # 65 (edge features + ones row for bias)
NK = ND * ND                  # 1024 A-columns
F32 = mybir.dt.float32
F32R = mybir.dt.float32r
BF16 = mybir.dt.bfloat16

# Per-tile count of PSUM bank-pairs handled by the fused DVE
# scalar_tensor_tensor path (rest: ACT-relu + DVE-tensor_tensor).
STT_PAIRS = [1, 2]  # cycled by tile index


def _build_nc(repeat: int = 1):
    nc = bacc.Bacc("TRN2", target_bir_lowering=False, debug=False,
                   num_devices=N_CORES)
    edgeT_d = nc.dram_tensor("edgeT", [KDIM, E_CORE], F32R, kind="ExternalInput")
    nodeT_d = nc.dram_tensor("nodeT", [ND, E_CORE], BF16, kind="ExternalInput")
    w_d = nc.dram_tensor("w_aug", [KDIM, NK], F32R, kind="ExternalInput")
    sel_d = nc.dram_tensor("sel", [128, 8 * ND], BF16, kind="ExternalInput")
    out_d = nc.dram_tensor("msg_raw", [128, E_CORE // 4], F32,
                           kind="ExternalOutput")

    with tile.TileContext(nc) as tc:
        with (
            tc.tile_pool(name="const", bufs=1) as constp,
            tc.tile_pool(name="edge", bufs=3) as edgep,
            tc.tile_pool(name="node", bufs=3) as nodep,
            tc.tile_pool(name="ar", bufs=5) as arp,
            tc.tile_pool(name="pp", bufs=8) as ppp,
            tc.tile_pool(name="mo", bufs=2) as mop,
            tc.tile_pool(name="apsum", bufs=3, space="PSUM") as apsum,
            tc.tile_pool(name="mpsum", bufs=2, space="PSUM") as mpsum,
        ):
            w_sb = constp.tile([KDIM, NK], F32R, name="w_sb")
            # Startup criticals split across the two HWDGE queues so block g
            # of W and the first edge chunk land just-in-time for their
            # first Ldweights/Matmult (each queue dispatches serially at
            # ~650ns per DMA): SP takes W block 0, ACT takes edge chunk 0
            # (issued in load_group) and the remaining W chunks.
            w_chunks = [(0, 1), (1, 2), (2, 4), (4, 6), (6, 8)]
            sel_sb = constp.tile([128, 8 * ND], BF16, name="sel_sb")
            sel_loaded = False

            # selector-matmul jobs lag one full tile behind the mm1 stream:
            # the PE is in-order, so a sel MM issued right after its pp is
            # produced stalls the array on the DVE/ACT latency. Each entry:
            # (mg_strip_ap, sel_block, pp, half, start, stop).
            sel_jobs = []

            def issue_sel(n):
                for _ in range(n):
                    if not sel_jobs:
                        return
                    mg_ap, blk, pp_, half, st, sp = sel_jobs.pop(0)
                    mg_t, c4_, chunk, tail = mg_ap
                    nc.tensor.matmul(mg_t[32 * c4_:32 * (c4_ + 1), :],
                                     sel_sb[:, ts(blk, ND)],
                                     pp_[:, ts(half, ET)],
                                     start=st, stop=sp,
                                     skip_group_check=True,
                                     tile_position=(0, 32 * c4_))
                    if sp and tail:
                        # final bank: evict + stream each strip as it lands
                        mo = mop.tile([32, ET], F32, name="mo_s")
                        nc.scalar.copy(mo[:], mg_t[32 * c4_:32 * (c4_ + 1), :])
                        nc.sync.dma_start(
                            out=out_d[32 * c4_:32 * (c4_ + 1), ts(chunk, ET)],
                            in_=mo[:])
                    elif sp and c4_ == 3:
                        # full 128-row bank: one evict + one DMA per 4 tiles
                        mo = mop.tile([128, ET], F32, name="mo")
                        nc.scalar.copy(mo[:], mg_t[:])
                        nc.gpsimd.dma_start(out=out_d[:, ts(chunk, ET)],
                                              in_=mo[:])

            def load_group(grp):
                # stream a 4-tile group of inputs; returns (ed_sb, nd_sb)
                gcols = ts(grp, GRP)
                ed_sb = edgep.tile([KDIM, GRP], F32R, name="ed_sb")
                nd_sb = nodep.tile([128, GRP], BF16, name="nd_sb")
                if grp == 0:
                    # startup: edge chunk 0 first on SP; W chunks ride the
                    # ACT queue so both queues dispatch in parallel
                    nc.sync.dma_start(out=ed_sb[:, ts(0, ET)],
                                      in_=edgeT_d[:, ts(0, ET)])
                    for lo, hi in w_chunks:
                        nc.scalar.dma_start(out=w_sb[:, 128 * lo:128 * hi],
                                            in_=w_d[:, 128 * lo:128 * hi])
                    for c in range(4):
                        nc.sync.dma_start(
                            out=nd_sb[32 * c:32 * (c + 1), :],
                            in_=nodeT_d[:, gcols])
                    for cc in range(1, GT):
                        nc.sync.dma_start(
                            out=ed_sb[:, ts(cc, ET)],
                            in_=edgeT_d[:, ts(cc, ET)])
                else:
                    nc.sync.dma_start(out=ed_sb[:], in_=edgeT_d[:, gcols])
                    for c in range(4):
                        nc.sync.dma_start(
                            out=nd_sb[32 * c:32 * (c + 1), :],
                            in_=nodeT_d[:, gcols])
                return ed_sb, nd_sb

            mg = None
            ngrp = NT // GT
            pending_grp = None
            for tg in range(NT * repeat):
                t = tg % NT
                grp, loc = divmod(t, GT)
                if tg == 0:
                    ed_sb, nd_sb = load_group(0)
                elif loc == 0:
                    ed_sb, nd_sb = pending_grp
                if loc == 1 and tg - loc + GT < NT * repeat:
                    # prefetch the next group 3 tiles ahead of first use
                    pending_grp = load_group((grp + 1) % ngrp)
                lcols = ts(loc, ET)
                if not sel_loaded:
                    # sel is first needed after the first fused pair; loading
                    # it after group 0 keeps the critical DMAs in front
                    nc.sync.dma_start(out=sel_sb[:], in_=sel_d[:])
                    sel_loaded = True

                # msg strip for this tile: rows 32c of the shared 4-tile bank
                c4 = t % 4
                if c4 == 0:
                    mg = mpsum.tile([128, ET], F32, name="mg")
                mg_ref = (mg, c4, t // 4, tg >= NT * repeat - 4)
                n_stt = 2 if tg == 0 else STT_PAIRS[t % len(STT_PAIRS)]
                # spread the DVE-heavy fused pairs across the tile
                stt_q = {0: (), 1: (1,), 2: (1, 3), 3: (0, 1, 3),
                         4: (0, 1, 2, 3)}[n_stt]
                new_jobs = []
                for q in range(4):
                    ap_t = apsum.tile([128, 2 * ET], F32, name="ap_t")
                    for gl in range(2):
                        g = 2 * q + gl
                        # float32r: fp32 operands, single-pass (relaxed
                        # precision) PE mode — 4x faster than strict fp32
                        nc.tensor.matmul(ap_t[:, ts(gl, ET)],
                                         w_sb[:, ts(g, 128)],
                                         ed_sb[:, lcols],
                                         start=True, stop=True)
                    pp = ppp.tile([128, 2 * ET], BF16, name="pp")
                    nd_b = nd_sb[:, lcols].unsqueeze(1).broadcast_to(
                        [128, 2, ET])
                    if q in stt_q:
                        # fused relu+mult straight from PSUM on DVE
                        nc.vector.scalar_tensor_tensor(
                            out=pp[:].rearrange("p (g e) -> p g e", g=2),
                            in0=ap_t[:].rearrange("p (g e) -> p g e", g=2),
                            scalar=0.0,
                            in1=nd_b,
                            op0=mybir.AluOpType.max,
                            op1=mybir.AluOpType.mult,
                        )
                    else:
                        # relu on ACT (PSUM->SBUF bf16), multiply on DVE at 2x
                        ar = arp.tile([128, 2 * ET], BF16, name="ar")
                        nc.scalar.activation(
                            ar[:], ap_t[:], mybir.ActivationFunctionType.Relu)
                        nc.vector.tensor_tensor(
                            out=pp[:].rearrange("p (g e) -> p g e", g=2),
                            in0=ar[:].rearrange("p (g e) -> p g e", g=2),
                            in1=nd_b,
                            op=mybir.AluOpType.mult,
                        )
                    new_jobs.append((mg_ref, 2 * q, pp, 0,
                                     q == 0, False))
                    new_jobs.append((mg_ref, 2 * q + 1, pp, 1,
                                     False, q == 3))
                    # interleave earlier tiles' sel MMs with this tile's
                    # mm1 pairs (2 per pair keeps PE fed); tile 1 issues
                    # only half, packed late (tile 0's pps are still in
                    # flight), priming a ~1.5-tile lag that rides out
                    # DVE/ACT latency bursts
                    issue_sel((0, 0, 1, 3)[q] if tg == 1 else 2)
                sel_jobs.extend(new_jobs)

            # drain the last tile's selector jobs
            issue_sel(len(sel_jobs))

    nc.compile()
    return nc


def _sel_matrix() -> np.ndarray:
    """sel[p, 32*g + m] = 1 iff m == 8*(g//2) + 4*(g%2) + p//32.

    Bank g holds A-columns k = 128g + p -> (i, j) = (4g + p//32, p%32).
    Selector block g routes its partitions to output row m = i (mod 32):
    i = 4g + p//32 = 8*(g//2) + 4*(g%2) + p//32."""
    sel = np.zeros((128, 8 * ND), dtype=np.float32)
    p = np.arange(128)
    for g in range(8):
        m = 8 * (g // 2) + 4 * (g % 2) + p // 32
        sel[p, 32 * g + m] = 1.0
    return sel.astype(ml_dtypes.bfloat16)


_LOCK = threading.Lock()
_NC = None


def _get_nc():
    global _NC
    with _LOCK:
        if _NC is None:
            _NC = _build_nc()
    return _NC


def _prep_inputs(node_j, edge_ij, W, b):
    node_j = np.asarray(node_j, dtype=np.float32)
    edge_ij = np.asarray(edge_ij, dtype=np.float32)
    W = np.asarray(W, dtype=np.float32)
    b = np.asarray(b, dtype=np.float32)

    edge_flat = edge_ij.reshape(EDGES, ED)
    edgeT_aug = np.empty((KDIM, EDGES), dtype=np.float32)
    edgeT_aug[:ED] = edge_flat.T
    edgeT_aug[ED] = 1.0

    nodeT = np.ascontiguousarray(
        node_j.reshape(EDGES, ND).T).astype(ml_dtypes.bfloat16)

    w_aug = np.empty((KDIM, NK), dtype=np.float32)
    w_aug[:ED] = W
    w_aug[ED] = b

    sel = _sel_matrix()

    in_maps = []
    for c in range(N_CORES):
        cols = slice(c * E_CORE, (c + 1) * E_CORE)
        in_maps.append({
            "edgeT": np.ascontiguousarray(edgeT_aug[:, cols]),
            "nodeT": np.ascontiguousarray(nodeT[:, cols]),
            "w_aug": w_aug,
            "sel": sel,
        })
    return in_maps


def _extract_msgT(msg_raw: np.ndarray) -> np.ndarray:
    """[128, E_core/4] packed 4-tile bank image -> msgT [32, E_core].

    Chunk k cols [512k:512(k+1)) hold tiles 4k..4k+3: strip c rows
    [32c:32c+32) are msg rows i of tile 4k+c."""
    nchunks = msg_raw.shape[1] // ET
    out = np.empty((ND, nchunks * 4 * ET), dtype=msg_raw.dtype)
    for k in range(nchunks):
        for c in range(4):
            tcols = slice((4 * k + c) * ET, (4 * k + c + 1) * ET)
            out[:, tcols] = msg_raw[32 * c:32 * (c + 1),
                                    k * ET:(k + 1) * ET]
    return out


def kernel(node_j, edge_ij, W, b):
    nc = _get_nc()
    in_maps = _prep_inputs(node_j, edge_ij, W, b)
    res = run_bass_kernel_spmd(nc, in_maps, core_ids=list(range(N_CORES)))
    msgT = np.concatenate(
        [_extract_msgT(res.results[c]["msg_raw"]) for c in range(N_CORES)],
        axis=1)  # [32, EDGES]
    return np.ascontiguousarray(msgT.T).reshape(B, E_FULL, ND)


# revision 27
# speedup vs baseline: 1.2333x; 1.2333x over previous
"""Trainium2 Bass kernel for nn_Message_Passer (gnn_message_passing).

Reference computation:
    A = relu(edge_ij @ W + b)            # [B, E, 1024]
    messages = einsum("beij,bej->bei", A.reshape(B,E,32,32), node_j)

Strategy (8 NeuronCores, data-parallel over the flattened B*E edge dim):
  - Host pre-transposes inputs: edgeT_aug [65, BE] (64 edge features + ones row
    so the bias rides inside the matmul), nodeT [32, BE] (bf16), W_aug [65,1024].
  - matmul1 (PE, float32r single-pass mode): lhsT = W_aug column-block g,
    rhs = edgeT tile -> AT_g [128, ET] in PSUM. Partition p of bank g is
    A-column k = 128g + p, i.e. (i, j) = (k // 32, k % 32).
  - Fused relu+multiply: P = max(AT, 0) * nodeT_rep, where nodeT_rep[p, e] =
    node[e, p % 32] (a 4x-replicated [128, *] bf16 tile serves every bank).
    Done with DVE scalar_tensor_tensor straight out of PSUM; a fraction of
    bank-pairs instead goes ACT relu (PSUM->SBUF bf16) + DVE tensor_tensor at
    2x so the work splits across both engines.
  - j-reduction via PE: constant 0/1 selector matmuls accumulate
    sum_j P[(i,j), e] into PSUM. Selector block (q, d) maps bank g = 2q+d
    partitions p to output row 32c + 8q + 4d + p//32 = 32c + i, where
    c = tile%4 selects the strip via tile_position, so FOUR tiles pack one
    msg PSUM bank [128, ET] fully (row 32c + i <-> tile 4k+c, msg row i).
  - One PSUM->SBUF msg eviction + one DMA per 4 tiles (full 128-row banks,
    4x less evict work and 4x less output DMA than per-tile strips).
  - Host extracts msg[tile 4k+c][e, i] = raw_k[32c + i, e].
"""

import threading

import numpy as np
import ml_dtypes

import concourse.bass as bass
import concourse.mybir as mybir
import concourse.tile as tile
from concourse import bacc
from concourse.bass import ts, ds
from concourse.bass_utils import run_bass_kernel_spmd

N_CORES = 8
B, E_FULL, ND, ED = 16, 4096, 32, 64
EDGES = B * E_FULL            # 65536
E_CORE = EDGES // N_CORES     # 8192
ET = 512                      # edges per on-chip tile
NT = E_CORE // ET             # 16 tiles
GT = 4                        # tiles per input-load group
GRP = GT * ET                 # 2048 edges per load group
KDIM = ED + 1                 # 65 (edge features + ones row for bias)
NK = ND * ND                  # 1024 A-columns
F32 = mybir.dt.float32
F32R = mybir.dt.float32r
BF16 = mybir.dt.bfloat16

APB, MPB = 3, 2

# Per-tile engine assignment for the relu(+mult) of the 4 PSUM bank-pairs:
# 'dve' = fused relu*mult STT on DVE (1x from PSUM); 'act' = relu on ACT
# (PSUM->SBUF bf16, GPSIMD cannot read PSUM) then 2x tensor_tensor mult on
# DVE; 'actp' = ACT relu then the mult on the otherwise-idle GPSIMD engine.
# Balanced so PE stays the sole bottleneck.
PAIR_MODES = [("dve", "act", "actp", "act"),
              ("dve", "act", "actp", "dve"),
              ("dve", "act", "actp", "act"),
              ("act", "dve", "actp", "act")]  # cycled by tile index


def _build_nc(repeat: int = 1):
    nc = bacc.Bacc("TRN2", target_bir_lowering=False, debug=False,
                   num_devices=N_CORES)
    edgeT_d = nc.dram_tensor("edgeT", [KDIM, E_CORE], F32R, kind="ExternalInput")
    nodeT_d = nc.dram_tensor("nodeT", [ND, E_CORE], BF16, kind="ExternalInput")
    w_d = nc.dram_tensor("w_aug", [KDIM, NK], F32R, kind="ExternalInput")
    sel_d = nc.dram_tensor("sel", [128, 8 * ND], BF16, kind="ExternalInput")
    out_d = nc.dram_tensor("msg_raw", [128, E_CORE // 4], F32,
                           kind="ExternalOutput")

    with tile.TileContext(nc) as tc:
        with (
            tc.tile_pool(name="const", bufs=1) as constp,
            tc.tile_pool(name="edge", bufs=3) as edgep,
            tc.tile_pool(name="node", bufs=3) as nodep,
            tc.tile_pool(name="ar", bufs=5) as arp,
            tc.tile_pool(name="pp", bufs=8) as ppp,
            tc.tile_pool(name="mo", bufs=2) as mop,
            tc.tile_pool(name="apsum", bufs=APB, space="PSUM") as apsum,
            tc.tile_pool(name="mpsum", bufs=MPB, space="PSUM") as mpsum,
        ):
            w_sb = constp.tile([KDIM, NK], F32R, name="w_sb")
            # Startup criticals split across the two HWDGE queues so block g
            # of W and the first edge chunk land just-in-time for their
            # first Ldweights/Matmult (each queue dispatches serially at
            # ~650ns per DMA): SP takes W block 0, ACT takes edge chunk 0
            # (issued in load_group) and the remaining W chunks.
            w_chunks = [(0, 1), (1, 2), (2, 4), (4, 6), (6, 8)]
            sel_sb = constp.tile([128, 8 * ND], BF16, name="sel_sb")
            sel_loaded = False

            # selector-matmul jobs lag one full tile behind the mm1 stream:
            # the PE is in-order, so a sel MM issued right after its pp is
            # produced stalls the array on the DVE/ACT latency. Each entry:
            # (mg_strip_ap, sel_block, pp, half, start, stop).
            sel_jobs = []

            def issue_sel(n):
                for _ in range(n):
                    if not sel_jobs:
                        return
                    mg_ap, blk, pp_, half, st, sp = sel_jobs.pop(0)
                    mg_t, c4_, chunk, tail = mg_ap
                    nc.tensor.matmul(mg_t[32 * c4_:32 * (c4_ + 1), :],
                                     sel_sb[:, ts(blk, ND)],
                                     pp_[:, ts(half, ET)],
                                     start=st, stop=sp,
                                     skip_group_check=True,
                                     tile_position=(0, 32 * c4_))
                    if sp and tail:
                        # final bank: evict + stream each strip as it lands
                        mo = mop.tile([32, ET], F32, name="mo_s")
                        nc.scalar.copy(mo[:], mg_t[32 * c4_:32 * (c4_ + 1), :])
                        nc.sync.dma_start(
                            out=out_d[32 * c4_:32 * (c4_ + 1), ts(chunk, ET)],
                            in_=mo[:])
                    elif sp and c4_ == 3:
                        # full 128-row bank: one evict + one DMA per 4 tiles
                        mo = mop.tile([128, ET], F32, name="mo")
                        nc.scalar.copy(mo[:], mg_t[:])
                        nc.gpsimd.dma_start(out=out_d[:, ts(chunk, ET)],
                                              in_=mo[:])

            def load_group(grp, first=False):
                # stream a 4-tile group of inputs; returns (ed_sb, nd_sb)
                gcols = ts(grp, GRP)
                ed_sb = edgep.tile([KDIM, GRP], F32R, name="ed_sb")
                nd_sb = nodep.tile([128, GRP], BF16, name="nd_sb")
                if first:
                    # startup: edge chunk 0 first on SP; W chunks ride the
                    # ACT queue so both queues dispatch in parallel
                    nc.sync.dma_start(out=ed_sb[:, ts(0, ET)],
                                      in_=edgeT_d[:, ts(0, ET)])
                    for lo, hi in w_chunks:
                        nc.scalar.dma_start(out=w_sb[:, 128 * lo:128 * hi],
                                            in_=w_d[:, 128 * lo:128 * hi])
                    for c in range(4):
                        nc.sync.dma_start(
                            out=nd_sb[32 * c:32 * (c + 1), :],
                            in_=nodeT_d[:, gcols])
                    for cc in range(1, GT):
                        nc.sync.dma_start(
                            out=ed_sb[:, ts(cc, ET)],
                            in_=edgeT_d[:, ts(cc, ET)])
                else:
                    nc.sync.dma_start(out=ed_sb[:], in_=edgeT_d[:, gcols])
                    for c in range(4):
                        nc.sync.dma_start(
                            out=nd_sb[32 * c:32 * (c + 1), :],
                            in_=nodeT_d[:, gcols])
                return ed_sb, nd_sb

            mg = None
            ngrp = NT // GT
            pending_grp = None
            for tg in range(NT * repeat):
                t = tg % NT
                grp, loc = divmod(t, GT)
                if tg == 0:
                    ed_sb, nd_sb = load_group(0, first=True)
                elif loc == 0:
                    ed_sb, nd_sb = pending_grp
                if loc == 1 and tg - loc + GT < NT * repeat:
                    # prefetch the next group 3 tiles ahead of first use
                    pending_grp = load_group((grp + 1) % ngrp)
                lcols = ts(loc, ET)
                if not sel_loaded:
                    # sel is first needed after the first fused pair; loading
                    # it after group 0 keeps the critical DMAs in front
                    nc.sync.dma_start(out=sel_sb[:], in_=sel_d[:])
                    sel_loaded = True

                # msg strip for this tile: rows 32c of the shared 4-tile bank
                c4 = t % 4
                if c4 == 0:
                    mg = mpsum.tile([128, ET], F32, name="mg")
                mg_ref = (mg, c4, t // 4, tg >= NT * repeat - 4)
                if tg == 0:
                    # ACT is still loading its activation table
                    modes = ("dve", "act", "dve", "dve")
                else:
                    modes = PAIR_MODES[t % len(PAIR_MODES)]
                new_jobs = []
                pool_jobs = []
                for q in range(4):
                    ap_t = apsum.tile([128, 2 * ET], F32, name="ap_t")
                    for gl in range(2):
                        g = 2 * q + gl
                        # float32r: fp32 operands, single-pass (relaxed
                        # precision) PE mode — 4x faster than strict fp32
                        nc.tensor.matmul(ap_t[:, ts(gl, ET)],
                                         w_sb[:, ts(g, 128)],
                                         ed_sb[:, lcols],
                                         start=True, stop=True)
                    pp = ppp.tile([128, 2 * ET], BF16, name="pp")
                    nd_b = nd_sb[:, lcols].unsqueeze(1).broadcast_to(
                        [128, 2, ET])
                    if modes[q] == "dve":
                        # fused relu+mult straight from PSUM on DVE
                        nc.vector.scalar_tensor_tensor(
                            out=pp[:].rearrange("p (g e) -> p g e", g=2),
                            in0=ap_t[:].rearrange("p (g e) -> p g e", g=2),
                            scalar=0.0,
                            in1=nd_b,
                            op0=mybir.AluOpType.max,
                            op1=mybir.AluOpType.mult,
                        )
                    else:
                        # relu on ACT (PSUM->SBUF bf16), multiply at 2x on
                        # DVE or (slower, but otherwise idle) GPSIMD
                        ar = arp.tile([128, 2 * ET], BF16, name="ar")
                        nc.scalar.activation(
                            ar[:], ap_t[:], mybir.ActivationFunctionType.Relu)
                        meng = nc.gpsimd if modes[q] == "actp" else nc.vector
                        meng.tensor_tensor(
                            out=pp[:].rearrange("p (g e) -> p g e", g=2),
                            in0=ar[:].rearrange("p (g e) -> p g e", g=2),
                            in1=nd_b,
                            op=mybir.AluOpType.mult,
                        )
                    dst = pool_jobs if modes[q] == "actp" else new_jobs
                    dst.append([mg_ref, 2 * q, pp, 0, False, False])
                    dst.append([mg_ref, 2 * q + 1, pp, 1, False, False])
                    # interleave earlier tiles' sel MMs with this tile's
                    # mm1 pairs (2 per pair keeps PE fed); tile 1 issues
                    # only half, packed late (tile 0's pps are still in
                    # flight), priming a ~1.5-tile lag that rides out
                    # DVE/ACT latency bursts
                    issue_sel((0, 0, 1, 3)[q] if tg == 1 else 2)
                # pool-produced pps arrive latest: issue their sel MMs
                # last so the in-order PE never waits on them mid-strip
                tile_jobs = new_jobs + pool_jobs
                tile_jobs[0][4] = True    # start accumulation on first issue
                tile_jobs[-1][5] = True   # stop on last issue
                sel_jobs.extend(tuple(j) for j in tile_jobs)

            # drain the last tile's selector jobs
            issue_sel(len(sel_jobs))

    nc.compile()
    return nc


def _sel_matrix() -> np.ndarray:
    """sel[p, 32*g + m] = 1 iff m == 8*(g//2) + 4*(g%2) + p//32.

    Bank g holds A-columns k = 128g + p -> (i, j) = (4g + p//32, p%32).
    Selector block g routes its partitions to output row m = i (mod 32):
    i = 4g + p//32 = 8*(g//2) + 4*(g%2) + p//32."""
    sel = np.zeros((128, 8 * ND), dtype=np.float32)
    p = np.arange(128)
    for g in range(8):
        m = 8 * (g // 2) + 4 * (g % 2) + p // 32
        sel[p, 32 * g + m] = 1.0
    return sel.astype(ml_dtypes.bfloat16)


_LOCK = threading.Lock()
_NC = None


def _get_nc():
    global _NC
    with _LOCK:
        if _NC is None:
            _NC = _build_nc()
    return _NC


def _prep_inputs(node_j, edge_ij, W, b):
    node_j = np.asarray(node_j, dtype=np.float32)
    edge_ij = np.asarray(edge_ij, dtype=np.float32)
    W = np.asarray(W, dtype=np.float32)
    b = np.asarray(b, dtype=np.float32)

    edge_flat = edge_ij.reshape(EDGES, ED)
    edgeT_aug = np.empty((KDIM, EDGES), dtype=np.float32)
    edgeT_aug[:ED] = edge_flat.T
    edgeT_aug[ED] = 1.0

    nodeT = np.ascontiguousarray(
        node_j.reshape(EDGES, ND).T).astype(ml_dtypes.bfloat16)

    w_aug = np.empty((KDIM, NK), dtype=np.float32)
    w_aug[:ED] = W
    w_aug[ED] = b

    sel = _sel_matrix()

    in_maps = []
    for c in range(N_CORES):
        cols = slice(c * E_CORE, (c + 1) * E_CORE)
        in_maps.append({
            "edgeT": np.ascontiguousarray(edgeT_aug[:, cols]),
            "nodeT": np.ascontiguousarray(nodeT[:, cols]),
            "w_aug": w_aug,
            "sel": sel,
        })
    return in_maps


def _extract_msgT(msg_raw: np.ndarray) -> np.ndarray:
    """[128, E_core/4] packed 4-tile bank image -> msgT [32, E_core].

    Chunk k cols [512k:512(k+1)) hold tiles 4k..4k+3: strip c rows
    [32c:32c+32) are msg rows i of tile 4k+c."""
    nchunks = msg_raw.shape[1] // ET
    out = np.empty((ND, nchunks * 4 * ET), dtype=msg_raw.dtype)
    for k in range(nchunks):
        for c in range(4):
            tcols = slice((4 * k + c) * ET, (4 * k + c + 1) * ET)
            out[:, tcols] = msg_raw[32 * c:32 * (c + 1),
                                    k * ET:(k + 1) * ET]
    return out


def kernel(node_j, edge_ij, W, b):
    nc = _get_nc()
    in_maps = _prep_inputs(node_j, edge_ij, W, b)
    res = run_bass_kernel_spmd(nc, in_maps, core_ids=list(range(N_CORES)))
    msgT = np.concatenate(
        [_extract_msgT(res.results[c]["msg_raw"]) for c in range(N_CORES)],
        axis=1)  # [32, EDGES]
    return np.ascontiguousarray(msgT.T).reshape(B, E_FULL, ND)


# revision 31
# speedup vs baseline: 1.8871x; 1.5302x over previous
"""Trainium2 Bass kernel for nn_Message_Passer (gnn_message_passing).

Reference computation:
    A = relu(edge_ij @ W + b)            # [B, E, 1024]
    messages = einsum("beij,bej->bei", A.reshape(B,E,32,32), node_j)

Strategy (8 NeuronCores, data-parallel over the flattened B*E edge dim):
  - Host pre-transposes inputs: edgeT_aug [65, BE] (64 edge features + ones row
    so the bias rides inside the matmul), nodeT [32, BE] (bf16), W_aug [65,1024].
  - matmul1 (PE, float32r single-pass mode): lhsT = W_aug column-block g,
    rhs = edgeT tile -> AT_g [128, ET] in PSUM. Partition p of bank g is
    A-column k = 128g + p, i.e. (i, j) = (k // 32, k % 32).
  - Fused relu+multiply: P = max(AT, 0) * nodeT_rep, where nodeT_rep[p, e] =
    node[e, p % 32] (a 4x-replicated [128, *] bf16 tile serves every bank).
    Done with DVE scalar_tensor_tensor straight out of PSUM; a fraction of
    bank-pairs instead goes ACT relu (PSUM->SBUF bf16) + DVE tensor_tensor at
    2x so the work splits across both engines.
  - j-reduction via PE: constant 0/1 selector matmuls accumulate
    sum_j P[(i,j), e] into PSUM. Selector block (q, d) maps bank g = 2q+d
    partitions p to output row 32c + 8q + 4d + p//32 = 32c + i, where
    c = tile%4 selects the strip via tile_position, so FOUR tiles pack one
    msg PSUM bank [128, ET] fully (row 32c + i <-> tile 4k+c, msg row i).
  - One PSUM->SBUF msg eviction + one DMA per 4 tiles (full 128-row banks,
    4x less evict work and 4x less output DMA than per-tile strips).
  - Host extracts msg[tile 4k+c][e, i] = raw_k[32c + i, e].
"""

import threading

import numpy as np
import ml_dtypes

import concourse.bass as bass
import concourse.mybir as mybir
import concourse.tile as tile
from concourse import bacc
from concourse.bass import ts, ds
from concourse.bass_utils import run_bass_kernel_spmd

N_CORES = 8
B, E_FULL, ND, ED = 16, 4096, 32, 64
EDGES = B * E_FULL            # 65536
E_CORE = EDGES // N_CORES     # 8192
ET = 512                      # edges per on-chip tile
NT = E_CORE // ET             # 16 tiles
GT = 4                        # tiles per input-load group
GRP = GT * ET                 # 2048 edges per load group
KDIM = ED + 1                 # 65 (edge features + ones row for bias)
NK = ND * ND                  # 1024 A-columns
F32 = mybir.dt.float32
F32R = mybir.dt.float32r
BF16 = mybir.dt.bfloat16

APB, MPB = 3, 2

# Per-tile engine assignment for the relu(+mult) of the 4 PSUM bank-pairs:
# 'dve' = fused relu*mult STT on DVE (1x from PSUM); 'act' = relu on ACT
# (PSUM->SBUF bf16, GPSIMD cannot read PSUM) then 2x tensor_tensor mult on
# DVE; 'actp' = ACT relu then the mult on the otherwise-idle GPSIMD engine.
# Balanced so PE stays the sole bottleneck.
PAIR_MODES = [("dve", "act", "actp", "act"),
              ("dve", "act", "actp", "dve"),
              ("dve", "act", "actp", "act"),
              ("act", "dve", "actp", "act")]  # cycled by tile index


def _build_nc(repeat: int = 1):
    nc = bacc.Bacc("TRN2", target_bir_lowering=False, debug=False,
                   num_devices=N_CORES)
    edgeT_d = nc.dram_tensor("edgeT", [KDIM, E_CORE], F32R, kind="ExternalInput")
    nodeT_d = nc.dram_tensor("nodeT", [ND, E_CORE], BF16, kind="ExternalInput")
    w_d = nc.dram_tensor("w_aug", [KDIM, NK], F32R, kind="ExternalInput")
    sel_d = nc.dram_tensor("sel", [128, 8 * ND], BF16, kind="ExternalInput")
    out_d = nc.dram_tensor("msg_raw", [128, E_CORE // 4], F32,
                           kind="ExternalOutput")

    with tile.TileContext(nc) as tc:
        with (
            tc.tile_pool(name="const", bufs=1) as constp,
            tc.tile_pool(name="edge", bufs=3) as edgep,
            tc.tile_pool(name="node", bufs=3) as nodep,
            tc.tile_pool(name="ar", bufs=5) as arp,
            tc.tile_pool(name="pp", bufs=8) as ppp,
            tc.tile_pool(name="mo", bufs=2) as mop,
            tc.tile_pool(name="apsum", bufs=APB, space="PSUM") as apsum,
            tc.tile_pool(name="mpsum", bufs=MPB, space="PSUM") as mpsum,
        ):
            w_sb = constp.tile([KDIM, NK], F32R, name="w_sb")
            # Startup criticals split across the two HWDGE queues so block g
            # of W and the first edge chunk land just-in-time for their
            # first Ldweights/Matmult (each queue dispatches serially at
            # ~650ns per DMA): SP takes W block 0, ACT takes edge chunk 0
            # (issued in load_group) and the remaining W chunks.
            w_chunks = [(0, 1), (1, 2), (2, 4), (4, 6), (6, 8)]
            sel_sb = constp.tile([128, 8 * ND], BF16, name="sel_sb")
            sel_loaded = False

            # selector-matmul jobs lag one full tile behind the mm1 stream:
            # the PE is in-order, so a sel MM issued right after its pp is
            # produced stalls the array on the DVE/ACT latency. Each entry:
            # (mg_strip_ap, sel_block, pp, half, start, stop).
            sel_jobs = []

            def issue_sel(n):
                for _ in range(n):
                    if not sel_jobs:
                        return
                    mg_ap, blk, pp_, half, st, sp = sel_jobs.pop(0)
                    mg_t, c4_, chunk, tail = mg_ap
                    nc.tensor.matmul(mg_t[32 * c4_:32 * (c4_ + 1), :],
                                     sel_sb[:, ts(blk, ND)],
                                     pp_[:, ts(half, ET)],
                                     start=st, stop=sp,
                                     skip_group_check=True,
                                     tile_position=(0, 32 * c4_))
                    if sp and tail:
                        # final bank: evict + stream each strip as it lands
                        mo = mop.tile([32, ET], F32, name="mo_s")
                        nc.scalar.copy(mo[:], mg_t[32 * c4_:32 * (c4_ + 1), :])
                        nc.sync.dma_start(
                            out=out_d[32 * c4_:32 * (c4_ + 1), ts(chunk, ET)],
                            in_=mo[:])
                    elif sp and c4_ == 3:
                        # full 128-row bank: one evict + one DMA per 4 tiles
                        mo = mop.tile([128, ET], F32, name="mo")
                        nc.scalar.copy(mo[:], mg_t[:])
                        nc.gpsimd.dma_start(out=out_d[:, ts(chunk, ET)],
                                              in_=mo[:])

            def load_group(grp, first=False):
                # stream a 4-tile group of inputs; returns (ed_sb, nd_sb)
                gcols = ts(grp, GRP)
                ed_sb = edgep.tile([KDIM, GRP], F32R, name="ed_sb")
                nd_sb = nodep.tile([128, GRP], BF16, name="nd_sb")
                if first:
                    # startup: edge chunk 0 first on SP; W chunks ride the
                    # ACT queue so both queues dispatch in parallel
                    nc.sync.dma_start(out=ed_sb[:, ts(0, ET)],
                                      in_=edgeT_d[:, ts(0, ET)])
                    for lo, hi in w_chunks:
                        nc.scalar.dma_start(out=w_sb[:, 128 * lo:128 * hi],
                                            in_=w_d[:, 128 * lo:128 * hi])
                    for c in range(4):
                        nc.sync.dma_start(
                            out=nd_sb[32 * c:32 * (c + 1), :],
                            in_=nodeT_d[:, gcols])
                    for cc in range(1, GT):
                        nc.sync.dma_start(
                            out=ed_sb[:, ts(cc, ET)],
                            in_=edgeT_d[:, ts(cc, ET)])
                else:
                    nc.sync.dma_start(out=ed_sb[:], in_=edgeT_d[:, gcols])
                    for c in range(4):
                        nc.sync.dma_start(
                            out=nd_sb[32 * c:32 * (c + 1), :],
                            in_=nodeT_d[:, gcols])
                return ed_sb, nd_sb

            mg = None
            ngrp = NT // GT
            pending_grp = None
            for tg in range(NT * repeat):
                t = tg % NT
                grp, loc = divmod(t, GT)
                if tg == 0:
                    ed_sb, nd_sb = load_group(0, first=True)
                elif loc == 0:
                    ed_sb, nd_sb = pending_grp
                if loc == 1 and tg - loc + GT < NT * repeat:
                    # prefetch the next group 3 tiles ahead of first use
                    pending_grp = load_group((grp + 1) % ngrp)
                lcols = ts(loc, ET)
                if not sel_loaded:
                    # sel is first needed after the first fused pair; loading
                    # it after group 0 keeps the critical DMAs in front
                    nc.sync.dma_start(out=sel_sb[:], in_=sel_d[:])
                    sel_loaded = True

                # msg strip for this tile: rows 32c of the shared 4-tile bank
                c4 = t % 4
                if c4 == 0:
                    mg = mpsum.tile([128, ET], F32, name="mg")
                mg_ref = (mg, c4, t // 4, tg >= NT * repeat - 4)
                if tg == 0:
                    # ACT is still loading its activation table
                    modes = ("dve", "act", "dve", "dve")
                else:
                    modes = PAIR_MODES[t % len(PAIR_MODES)]
                new_jobs = []
                pool_jobs = []
                for q in range(4):
                    ap_t = apsum.tile([128, 2 * ET], F32, name="ap_t")
                    for gl in range(2):
                        g = 2 * q + gl
                        # float32r: fp32 operands, single-pass (relaxed
                        # precision) PE mode — 4x faster than strict fp32
                        nc.tensor.matmul(ap_t[:, ts(gl, ET)],
                                         w_sb[:, ts(g, 128)],
                                         ed_sb[:, lcols],
                                         start=True, stop=True)
                    pp = ppp.tile([128, 2 * ET], BF16, name="pp")
                    nd_b = nd_sb[:, lcols].unsqueeze(1).broadcast_to(
                        [128, 2, ET])
                    if modes[q] == "dve":
                        # fused relu+mult straight from PSUM on DVE
                        nc.vector.scalar_tensor_tensor(
                            out=pp[:].rearrange("p (g e) -> p g e", g=2),
                            in0=ap_t[:].rearrange("p (g e) -> p g e", g=2),
                            scalar=0.0,
                            in1=nd_b,
                            op0=mybir.AluOpType.max,
                            op1=mybir.AluOpType.mult,
                        )
                    else:
                        # relu on ACT (PSUM->SBUF bf16), multiply at 2x on
                        # DVE or (slower, but otherwise idle) GPSIMD
                        ar = arp.tile([128, 2 * ET], BF16, name="ar")
                        nc.scalar.activation(
                            ar[:], ap_t[:], mybir.ActivationFunctionType.Relu)
                        meng = nc.gpsimd if modes[q] == "actp" else nc.vector
                        meng.tensor_tensor(
                            out=pp[:].rearrange("p (g e) -> p g e", g=2),
                            in0=ar[:].rearrange("p (g e) -> p g e", g=2),
                            in1=nd_b,
                            op=mybir.AluOpType.mult,
                        )
                    dst = pool_jobs if modes[q] == "actp" else new_jobs
                    dst.append([mg_ref, 2 * q, pp, 0, False, False])
                    dst.append([mg_ref, 2 * q + 1, pp, 1, False, False])
                    # interleave earlier tiles' sel MMs with this tile's
                    # mm1 pairs (2 per pair keeps PE fed); tile 1 issues
                    # only half, packed late (tile 0's pps are still in
                    # flight), priming a ~1.5-tile lag that rides out
                    # DVE/ACT latency bursts
                    issue_sel((0, 0, 1, 3)[q] if tg == 1 else 2)
                # pool-produced pps arrive latest: issue their sel MMs
                # last so the in-order PE never waits on them mid-strip
                tile_jobs = new_jobs + pool_jobs
                tile_jobs[0][4] = True    # start accumulation on first issue
                tile_jobs[-1][5] = True   # stop on last issue
                sel_jobs.extend(tuple(j) for j in tile_jobs)

            # drain the last tile's selector jobs
            issue_sel(len(sel_jobs))

    nc.compile()
    return nc


def _sel_matrix() -> np.ndarray:
    """sel[p, 32*g + m] = 1 iff m == 8*(g//2) + 4*(g%2) + p//32.

    Bank g holds A-columns k = 128g + p -> (i, j) = (4g + p//32, p%32).
    Selector block g routes its partitions to output row m = i (mod 32):
    i = 4g + p//32 = 8*(g//2) + 4*(g%2) + p//32."""
    sel = np.zeros((128, 8 * ND), dtype=np.float32)
    p = np.arange(128)
    for g in range(8):
        m = 8 * (g // 2) + 4 * (g % 2) + p // 32
        sel[p, 32 * g + m] = 1.0
    return sel.astype(ml_dtypes.bfloat16)


_LOCK = threading.Lock()
_NC = None


def _get_nc():
    global _NC
    with _LOCK:
        if _NC is None:
            _NC = _build_nc()
    return _NC


def _prep_inputs(node_j, edge_ij, W, b):
    node_j = np.asarray(node_j, dtype=np.float32)
    edge_ij = np.asarray(edge_ij, dtype=np.float32)
    W = np.asarray(W, dtype=np.float32)
    b = np.asarray(b, dtype=np.float32)

    edge_flat = edge_ij.reshape(EDGES, ED)
    edgeT_aug = np.empty((KDIM, EDGES), dtype=np.float32)
    edgeT_aug[:ED] = edge_flat.T
    edgeT_aug[ED] = 1.0

    nodeT = np.ascontiguousarray(
        node_j.reshape(EDGES, ND).T).astype(ml_dtypes.bfloat16)

    w_aug = np.empty((KDIM, NK), dtype=np.float32)
    w_aug[:ED] = W
    w_aug[ED] = b

    sel = _sel_matrix()

    in_maps = []
    for c in range(N_CORES):
        cols = slice(c * E_CORE, (c + 1) * E_CORE)
        in_maps.append({
            "edgeT": np.ascontiguousarray(edgeT_aug[:, cols]),
            "nodeT": np.ascontiguousarray(nodeT[:, cols]),
            "w_aug": w_aug,
            "sel": sel,
        })
    return in_maps


def _extract_msgT(msg_raw: np.ndarray) -> np.ndarray:
    """[128, E_core/4] packed 4-tile bank image -> msgT [32, E_core].

    Chunk k cols [512k:512(k+1)) hold tiles 4k..4k+3: strip c rows
    [32c:32c+32) are msg rows i of tile 4k+c."""
    nchunks = msg_raw.shape[1] // ET
    out = np.empty((ND, nchunks * 4 * ET), dtype=msg_raw.dtype)
    for k in range(nchunks):
        for c in range(4):
            tcols = slice((4 * k + c) * ET, (4 * k + c + 1) * ET)
            out[:, tcols] = msg_raw[32 * c:32 * (c + 1),
                                    k * ET:(k + 1) * ET]
    return out


def kernel(node_j, edge_ij, W, b):
    nc = _get_nc()
    in_maps = _prep_inputs(node_j, edge_ij, W, b)
    res = run_bass_kernel_spmd(nc, in_maps, core_ids=list(range(N_CORES)))
    msgT = np.concatenate(
        [_extract_msgT(res.results[c]["msg_raw"]) for c in range(N_CORES)],
        axis=1)  # [32, EDGES]
    return np.ascontiguousarray(msgT.T).reshape(B, E_FULL, ND)


# revision 37
# speedup vs baseline: 1.9385x; 1.0272x over previous
"""Trainium2 Bass kernel for nn_Message_Passer (gnn_message_passing).

Reference computation:
    A = relu(edge_ij @ W + b)            # [B, E, 1024]
    messages = einsum("beij,bej->bei", A.reshape(B,E,32,32), node_j)

Strategy (8 NeuronCores, data-parallel over the flattened B*E edge dim):
  - Host pre-transposes inputs: edgeT_aug [65, BE] (64 edge features + ones row
    so the bias rides inside the matmul), nodeT [32, BE] (bf16), W_aug [65,1024].
  - matmul1 (PE, float32r single-pass mode): lhsT = W_aug column-block g,
    rhs = edgeT tile -> AT_g [128, ET] in PSUM. Partition p of bank g is
    A-column k = 128g + p, i.e. (i, j) = (k // 32, k % 32).
  - Fused relu+multiply: P = max(AT, 0) * nodeT_rep, where nodeT_rep[p, e] =
    node[e, p % 32] (a 4x-replicated [128, *] bf16 tile serves every bank).
    Done with DVE scalar_tensor_tensor straight out of PSUM; a fraction of
    bank-pairs instead goes ACT relu (PSUM->SBUF bf16) + DVE tensor_tensor at
    2x so the work splits across both engines.
  - j-reduction via PE: constant 0/1 selector matmuls accumulate
    sum_j P[(i,j), e] into PSUM. Selector block (q, d) maps bank g = 2q+d
    partitions p to output row 32c + 8q + 4d + p//32 = 32c + i, where
    c = tile%4 selects the strip via tile_position, so FOUR tiles pack one
    msg PSUM bank [128, ET] fully (row 32c + i <-> tile 4k+c, msg row i).
  - One PSUM->SBUF msg eviction + one DMA per 4 tiles (full 128-row banks,
    4x less evict work and 4x less output DMA than per-tile strips).
  - Host extracts msg[tile 4k+c][e, i] = raw_k[32c + i, e].
"""

import threading

import numpy as np
import ml_dtypes

import concourse.bass as bass
import concourse.mybir as mybir
import concourse.tile as tile
from concourse import bacc
from concourse.bass import ts, ds
from concourse.bass_utils import run_bass_kernel_spmd

N_CORES = 8
B, E_FULL, ND, ED = 16, 4096, 32, 64
EDGES = B * E_FULL            # 65536
E_CORE = EDGES // N_CORES     # 8192
ET = 512                      # edges per on-chip tile
NT = E_CORE // ET             # 16 tiles
GT = 4                        # tiles per input-load group
GRP = GT * ET                 # 2048 edges per load group
KDIM = ED + 1                 # 65 (edge features + ones row for bias)
NK = ND * ND                  # 1024 A-columns
F32 = mybir.dt.float32
F32R = mybir.dt.float32r
BF16 = mybir.dt.bfloat16

APB, MPB = 3, 2

# Per-tile engine assignment for the relu(+mult) of the 4 PSUM bank-pairs:
# 'dve' = fused relu*mult STT on DVE (1x from PSUM); 'act' = relu on ACT
# (PSUM->SBUF bf16, GPSIMD cannot read PSUM) then 2x tensor_tensor mult on
# DVE; 'actp' = ACT relu then the mult on the otherwise-idle GPSIMD engine.
# Balanced so PE stays the sole bottleneck.
PAIR_MODES = [("dve", "act", "actp", "act"),
              ("dve", "act", "actp", "dve"),
              ("dve", "act", "actp", "act"),
              ("act", "dve", "actp", "act")]  # cycled by tile index


def _build_nc(repeat: int = 1):
    nc = bacc.Bacc("TRN2", target_bir_lowering=False, debug=False,
                   num_devices=N_CORES)
    edgeT_d = nc.dram_tensor("edgeT", [KDIM, E_CORE], F32R, kind="ExternalInput")
    nodeT_d = nc.dram_tensor("nodeT", [ND, E_CORE], BF16, kind="ExternalInput")
    w_d = nc.dram_tensor("w_aug", [KDIM, NK], F32R, kind="ExternalInput")
    sel_d = nc.dram_tensor("sel", [128, 8 * ND], BF16, kind="ExternalInput")
    out_d = nc.dram_tensor("msg_raw", [128, E_CORE // 4], F32,
                           kind="ExternalOutput")

    with tile.TileContext(nc) as tc:
        with (
            tc.tile_pool(name="const", bufs=1) as constp,
            tc.tile_pool(name="edge", bufs=3) as edgep,
            tc.tile_pool(name="node", bufs=3) as nodep,
            tc.tile_pool(name="ar", bufs=6) as arp,
            tc.tile_pool(name="pp", bufs=10) as ppp,
            tc.tile_pool(name="mo", bufs=3) as mop,
            tc.tile_pool(name="apsum", bufs=APB, space="PSUM") as apsum,
            tc.tile_pool(name="mpsum", bufs=MPB, space="PSUM") as mpsum,
        ):
            w_sb = constp.tile([KDIM, NK], F32R, name="w_sb")
            # Startup criticals split across the two HWDGE queues so block g
            # of W and the first edge chunk land just-in-time for their
            # first Ldweights/Matmult (each queue dispatches serially at
            # ~650ns per DMA): SP takes W block 0, ACT takes edge chunk 0
            # (issued in load_group) and the remaining W chunks.
            w_chunks = [(0, 1), (1, 2), (2, 4), (4, 6), (6, 8)]
            sel_sb = constp.tile([128, 8 * ND], BF16, name="sel_sb")
            sel_loaded = False

            # selector-matmul jobs lag one full tile behind the mm1 stream:
            # the PE is in-order, so a sel MM issued right after its pp is
            # produced stalls the array on the DVE/ACT latency. Each entry:
            # (mg_strip_ap, sel_block, pp, half, start, stop).
            sel_jobs = []

            def issue_sel(n):
                for _ in range(n):
                    if not sel_jobs:
                        return
                    mg_ap, blk, pp_, half, st, sp = sel_jobs.pop(0)
                    mg_t, c4_, chunk, tail = mg_ap
                    nc.tensor.matmul(mg_t[32 * c4_:32 * (c4_ + 1), :],
                                     sel_sb[:, ts(blk, ND)],
                                     pp_[:, ts(half, ET)],
                                     start=st, stop=sp,
                                     skip_group_check=True,
                                     tile_position=(0, 32 * c4_))
                    if sp and tail:
                        # final bank: evict + stream each strip as it lands
                        mo = mop.tile([32, ET], F32, name="mo_s")
                        nc.scalar.copy(mo[:], mg_t[32 * c4_:32 * (c4_ + 1), :])
                        nc.sync.dma_start(
                            out=out_d[32 * c4_:32 * (c4_ + 1), ts(chunk, ET)],
                            in_=mo[:])
                    elif sp and c4_ == 3:
                        # full 128-row bank: one evict + one DMA per 4 tiles
                        mo = mop.tile([128, ET], F32, name="mo")
                        nc.scalar.copy(mo[:], mg_t[:])
                        nc.gpsimd.dma_start(out=out_d[:, ts(chunk, ET)],
                                              in_=mo[:])

            def load_group(grp, first=False):
                # stream a 4-tile group of inputs; returns (ed_sb, nd_sb)
                gcols = ts(grp, GRP)
                ed_sb = edgep.tile([KDIM, GRP], F32R, name="ed_sb")
                nd_sb = nodep.tile([128, GRP], BF16, name="nd_sb")
                if first:
                    # startup: edge chunk 0 first on SP; W chunks ride the
                    # ACT queue so both queues dispatch in parallel
                    nc.sync.dma_start(out=ed_sb[:, ts(0, ET)],
                                      in_=edgeT_d[:, ts(0, ET)])
                    for lo, hi in w_chunks:
                        nc.scalar.dma_start(out=w_sb[:, 128 * lo:128 * hi],
                                            in_=w_d[:, 128 * lo:128 * hi])
                    for c in range(4):
                        nc.sync.dma_start(
                            out=nd_sb[32 * c:32 * (c + 1), :],
                            in_=nodeT_d[:, gcols])
                    for cc in range(1, GT):
                        nc.sync.dma_start(
                            out=ed_sb[:, ts(cc, ET)],
                            in_=edgeT_d[:, ts(cc, ET)])
                else:
                    nc.sync.dma_start(out=ed_sb[:], in_=edgeT_d[:, gcols])
                    for c in range(4):
                        nc.sync.dma_start(
                            out=nd_sb[32 * c:32 * (c + 1), :],
                            in_=nodeT_d[:, gcols])
                return ed_sb, nd_sb

            mg = None
            ngrp = NT // GT
            pending_grp = None
            for tg in range(NT * repeat):
                t = tg % NT
                grp, loc = divmod(t, GT)
                if tg == 0:
                    ed_sb, nd_sb = load_group(0, first=True)
                elif loc == 0:
                    ed_sb, nd_sb = pending_grp
                if loc == 1 and tg - loc + GT < NT * repeat:
                    # prefetch the next group 3 tiles ahead of first use
                    pending_grp = load_group((grp + 1) % ngrp)
                lcols = ts(loc, ET)
                if not sel_loaded:
                    # sel is first needed after the first fused pair; loading
                    # it after group 0 keeps the critical DMAs in front
                    nc.sync.dma_start(out=sel_sb[:], in_=sel_d[:])
                    sel_loaded = True

                # msg strip for this tile: rows 32c of the shared 4-tile bank
                c4 = t % 4
                if c4 == 0:
                    mg = mpsum.tile([128, ET], F32, name="mg")
                mg_ref = (mg, c4, t // 4, tg >= NT * repeat - 4)
                if tg == 0:
                    # ACT is still loading its activation table
                    modes = ("dve", "act", "dve", "dve")
                else:
                    modes = PAIR_MODES[t % len(PAIR_MODES)]
                new_jobs = []
                pool_jobs = []
                for q in range(4):
                    ap_t = apsum.tile([128, 2 * ET], F32, name="ap_t")
                    for gl in range(2):
                        g = 2 * q + gl
                        # float32r: fp32 operands, single-pass (relaxed
                        # precision) PE mode — 4x faster than strict fp32
                        nc.tensor.matmul(ap_t[:, ts(gl, ET)],
                                         w_sb[:, ts(g, 128)],
                                         ed_sb[:, lcols],
                                         start=True, stop=True)
                    pp = ppp.tile([128, 2 * ET], BF16, name="pp")
                    nd_b = nd_sb[:, lcols].unsqueeze(1).broadcast_to(
                        [128, 2, ET])
                    if modes[q] == "dve":
                        # fused relu+mult straight from PSUM on DVE
                        nc.vector.scalar_tensor_tensor(
                            out=pp[:].rearrange("p (g e) -> p g e", g=2),
                            in0=ap_t[:].rearrange("p (g e) -> p g e", g=2),
                            scalar=0.0,
                            in1=nd_b,
                            op0=mybir.AluOpType.max,
                            op1=mybir.AluOpType.mult,
                        )
                    else:
                        # relu on ACT (PSUM->SBUF bf16), multiply at 2x on
                        # DVE or (slower, but otherwise idle) GPSIMD
                        ar = arp.tile([128, 2 * ET], BF16, name="ar")
                        nc.scalar.activation(
                            ar[:], ap_t[:], mybir.ActivationFunctionType.Relu)
                        meng = nc.gpsimd if modes[q] == "actp" else nc.vector
                        meng.tensor_tensor(
                            out=pp[:].rearrange("p (g e) -> p g e", g=2),
                            in0=ar[:].rearrange("p (g e) -> p g e", g=2),
                            in1=nd_b,
                            op=mybir.AluOpType.mult,
                        )
                    dst = pool_jobs if modes[q] == "actp" else new_jobs
                    dst.append([mg_ref, 2 * q, pp, 0, False, False])
                    dst.append([mg_ref, 2 * q + 1, pp, 1, False, False])
                    # interleave earlier tiles' sel MMs with this tile's
                    # mm1 pairs (2 per pair keeps PE fed); tile 1 issues
                    # only half, packed late (tile 0's pps are still in
                    # flight), priming a ~1.5-tile lag that rides out
                    # DVE/ACT latency bursts
                    issue_sel((0, 0, 1, 3)[q] if tg == 1 else 2)
                # pool-produced pps arrive latest: issue their sel MMs
                # last so the in-order PE never waits on them mid-strip
                tile_jobs = new_jobs + pool_jobs
                tile_jobs[0][4] = True    # start accumulation on first issue
                tile_jobs[-1][5] = True   # stop on last issue
                sel_jobs.extend(tuple(j) for j in tile_jobs)

            # drain the last tile's selector jobs
            issue_sel(len(sel_jobs))

    nc.compile()
    return nc


def _sel_matrix() -> np.ndarray:
    """sel[p, 32*g + m] = 1 iff m == 8*(g//2) + 4*(g%2) + p//32.

    Bank g holds A-columns k = 128g + p -> (i, j) = (4g + p//32, p%32).
    Selector block g routes its partitions to output row m = i (mod 32):
    i = 4g + p//32 = 8*(g//2) + 4*(g%2) + p//32."""
    sel = np.zeros((128, 8 * ND), dtype=np.float32)
    p = np.arange(128)
    for g in range(8):
        m = 8 * (g // 2) + 4 * (g % 2) + p // 32
        sel[p, 32 * g + m] = 1.0
    return sel.astype(ml_dtypes.bfloat16)


_LOCK = threading.Lock()
_NC = None


def _get_nc():
    global _NC
    with _LOCK:
        if _NC is None:
            _NC = _build_nc()
    return _NC


def _prep_inputs(node_j, edge_ij, W, b):
    node_j = np.asarray(node_j, dtype=np.float32)
    edge_ij = np.asarray(edge_ij, dtype=np.float32)
    W = np.asarray(W, dtype=np.float32)
    b = np.asarray(b, dtype=np.float32)

    edge_flat = edge_ij.reshape(EDGES, ED)
    edgeT_aug = np.empty((KDIM, EDGES), dtype=np.float32)
    edgeT_aug[:ED] = edge_flat.T
    edgeT_aug[ED] = 1.0

    nodeT = np.ascontiguousarray(
        node_j.reshape(EDGES, ND).T).astype(ml_dtypes.bfloat16)

    w_aug = np.empty((KDIM, NK), dtype=np.float32)
    w_aug[:ED] = W
    w_aug[ED] = b

    sel = _sel_matrix()

    in_maps = []
    for c in range(N_CORES):
        cols = slice(c * E_CORE, (c + 1) * E_CORE)
        in_maps.append({
            "edgeT": np.ascontiguousarray(edgeT_aug[:, cols]),
            "nodeT": np.ascontiguousarray(nodeT[:, cols]),
            "w_aug": w_aug,
            "sel": sel,
        })
    return in_maps


def _extract_msgT(msg_raw: np.ndarray) -> np.ndarray:
    """[128, E_core/4] packed 4-tile bank image -> msgT [32, E_core].

    Chunk k cols [512k:512(k+1)) hold tiles 4k..4k+3: strip c rows
    [32c:32c+32) are msg rows i of tile 4k+c."""
    nchunks = msg_raw.shape[1] // ET
    out = np.empty((ND, nchunks * 4 * ET), dtype=msg_raw.dtype)
    for k in range(nchunks):
        for c in range(4):
            tcols = slice((4 * k + c) * ET, (4 * k + c + 1) * ET)
            out[:, tcols] = msg_raw[32 * c:32 * (c + 1),
                                    k * ET:(k + 1) * ET]
    return out


def kernel(node_j, edge_ij, W, b):
    nc = _get_nc()
    in_maps = _prep_inputs(node_j, edge_ij, W, b)
    res = run_bass_kernel_spmd(nc, in_maps, core_ids=list(range(N_CORES)))
    msgT = np.concatenate(
        [_extract_msgT(res.results[c]["msg_raw"]) for c in range(N_CORES)],
        axis=1)  # [32, EDGES]
    return np.ascontiguousarray(msgT.T).reshape(B, E_FULL, ND)
